# revision 28
# baseline (speedup 1.0000x reference)
"""Trainium2 Bass kernel for nn_BasicBlock (distance-transform conv BasicBlock).

Computes: relu(bn2(dt_conv2(relu(bn1(dt_conv1(x))))) + x)
where dt_conv is a 3x3 "distance transform conv":
    d[b,o,h,w] = sqrt(||p - c_o||^2),  p = 3x3 zero-padded patch (dim 576)

Strategy (8 NeuronCores, data-parallel over batch 32 -> 4 images/core):
- Image-PAIR packing: two images share the 128 PE partitions (img A on
  0:64, img B on 64:128) with block-diagonal weights, so every matmul
  column does 128 useful MACs (the old layout wasted half the array on
  M=64).
- fp8(e4m3) + MatmulPerfMode.DoubleRow at 0.5 cycles/row: the k-subtile
  dim packs TWO 3x3 offsets per pass via hand-built OVERLAPPING access
  patterns (j-dim stride = one plane row, or one column) -- zero shifted
  copies. 5 fp8 passes cover all 9 offsets (one j-slot zero-weighted).
- ||p||^2 via bf16 ones-passes: x^2 -> horizontal 3-sum h3 on ACT/DVE,
  then 3 vertical window passes on the PE with block-diag SC*ones.
  Keeping the squared term in bf16 (not fp8) bounds the d^2 error.
- All weights pre-scaled by SC=16 (fp8 dynamic range); eviction is
  d = sqrt(psum/SC + ||c||^2) on ACT with accum_out giving sum(d); DVE
  reduces psum for sum(d^2) concurrently. PSUM: [128,4,512] 4-bank
  tiles, double-buffered (all 8 banks).
- Engine balance: fp8 casts on Pool(gpsimd), squares/relu-affine on ACT,
  h-sums + psum-reduces + residual multiply-add on DVE (bf16 ops hit
  the DVE 2x mode).
- Sync-BN: [64,2] stats AllGather across 8 cores + local rank-sum
  (AllGather ~23us on HW; AllReduce measured 10x worse).
- Residual x is DMA'd once as contiguous 12.5KB/channel runs (16 DMA
  engines at full width) into an unpadded staging tile that doubles as
  the residual source; padded planes are built on compute engines.

kernel(**inputs) takes FULL unsharded inputs, returns FULL output.
Self-contained: shapes/sharding hardcoded; no file reads.
"""
import numpy as np
import ml_dtypes

from concourse import bacc, mybir, tile
from concourse.bass_utils import run_bass_kernel_spmd

f32 = mybir.dt.float32
bf16 = mybir.dt.bfloat16
fp8 = mybir.dt.float8e4
ADD = mybir.AluOpType.add
MULT = mybir.AluOpType.mult
SUB = mybir.AluOpType.subtract
AF = mybir.ActivationFunctionType
DR = mybir.MatmulPerfMode.DoubleRow

N_CORES = 8
B_LOCAL = 4            # images per core (32 / 8)
NPAIR = 2              # image pairs per core
C = 64                 # channels (in == out)
HW = 56                # spatial
HP = 58                # padded cols
PR = 32                # plane rows (padded half-image + halo/spare)
SC = 16.0              # fp8 weight scale; evict divides it back out
N_GLOBAL = 32 * HW * HW
BN_EPS = 1e-5

FUSE_GROUPS = 1   # PSUM groups (banks) per matmul instruction: 1, 2, or 4
CAST_ENGINE = "pool"   # engine for fp8 plane casts: pool | dve | act

# fp8 DoubleRow passes: (row_add, col_base, jstride_rows?, [(kh,kw) per j])
# j-dim is an overlapping view: stride = one row (HP elems) or one col (1).
FP8_PASSES = [
    (0, 0, True, [(0, 0), (1, 0)]),
    (0, 1, True, [(0, 1), (1, 1)]),
    (0, 2, True, [(0, 2), (1, 2)]),
    (2, 0, False, [(2, 0), (2, 1)]),
    (2, 2, True, [(2, 2), None]),
]


def _jpair(ap, jstride):
    """Overlapping k-subtile view: insert a j-dim (size 2, given stride)."""
    c = ap.copy()
    c.ap.insert(1, (jstride, 2))
    return c


def _layer(nc, psum, F8, H3, wx, wu, cst, ci, d, stats):
    """One dt_conv layer: per (pair q, half h): 4 groups x 8 passes.
    stats [128,8]: cols 0:4 = sum(d) per block, cols 4:8 = sum(psum)."""
    for q in range(NPAIR):
        for h in range(2):
            f8, h3 = F8[q][h], H3[q][h]
            ps = psum.tile([128, 4, 512], f32, tag="ps")
            ng = 4 // FUSE_GROUPS
            nr = 7 * FUSE_GROUPS     # fused groups are consecutive 7-row
            for gp in range(ng):     # blocks: (group,row) merges into one
                g = FUSE_GROUPS * gp  # taller window -- no extra AP dim
                l0 = 7 * g + (1 if h else 0)
                out = ps[:, g:g + FUSE_GROUPS, 0:392]
                for pi, (dr, cb, jrow, _offs) in enumerate(FP8_PASSES):
                    base = f8[:, l0 + dr:l0 + dr + nr, cb:cb + 56]
                    rhs = _jpair(base, HP if jrow else 1)
                    nc.tensor.matmul(out, wx[:, pi, :, :], rhs,
                                     start=(pi == 0), stop=False, perf_mode=DR)
                for kh in range(3):
                    nc.tensor.matmul(out, wu[:, :],
                                     h3[:, l0 + kh:l0 + kh + nr, 0:56],
                                     start=False, stop=(kh == 2))
            col = 2 * q + h
            rs = slice(h * 28, h * 28 + 28)
            # sum(psum) on DVE runs concurrently with the ACT eviction
            nc.vector.tensor_reduce(out=stats[:, 4 + col:5 + col],
                                    in_=ps[:, :, 0:392],
                                    axis=mybir.AxisListType.XY, op=ADD)
            nc.scalar.activation(out=d[:, q, rs, :], in_=ps[:, :, 0:392],
                                 func=AF.Sqrt, bias=cst[:, ci:ci + 1],
                                 scale=1.0 / SC,
                                 accum_out=stats[:, col:col + 1])


def _bn_affine(nc, pool, gstats, c2, gamma, beta, eps, name, n_stat=None):
    """[sum(d), sum(psum)] (dup halves) -> scale s, shift t [128,1]."""
    P = 2 * C
    n_stat = n_stat or N_GLOBAL
    mu = pool.tile([P, 1], f32, tag=f"mu_{name}")
    ed2 = pool.tile([P, 1], f32, tag=f"ed2_{name}")
    nvar = pool.tile([P, 1], f32, tag=f"nvar_{name}")
    sd = pool.tile([P, 1], f32, tag=f"sd_{name}")
    inv = pool.tile([P, 1], f32, tag=f"inv_{name}")
    s = pool.tile([P, 1], f32, tag=f"s_{name}")
    st = pool.tile([P, 1], f32, tag=f"st_{name}")
    tt = pool.tile([P, 1], f32, tag=f"t_{name}")
    inv_n = 1.0 / float(n_stat)
    nc.vector.tensor_scalar_mul(out=mu[:, :], in0=gstats[:, 0:1], scalar1=inv_n)
    # E[d^2] = sum(psum)*inv_n/SC + ||c||^2
    nc.vector.scalar_tensor_tensor(out=ed2[:, :], in0=gstats[:, 1:2],
                                   scalar=inv_n / SC, in1=c2,
                                   op0=MULT, op1=ADD)
    # nvar = mu^2 - E[d^2];  sd = sqrt(-nvar + eps) via ACT scale=-1
    nc.vector.scalar_tensor_tensor(out=nvar[:, :], in0=mu[:, :],
                                   scalar=mu[:, 0:1], in1=ed2[:, :],
                                   op0=MULT, op1=SUB)
    nc.scalar.activation(out=sd[:, :], in_=nvar[:, :], func=AF.Sqrt,
                         bias=eps[:, 0:1], scale=-1.0)
    nc.vector.reciprocal(out=inv[:, :], in_=sd[:, :])
    nc.vector.tensor_tensor(out=s[:, :], in0=gamma, in1=inv[:, :], op=MULT)
    nc.vector.tensor_tensor(out=st[:, :], in0=mu[:, :], in1=s[:, :], op=MULT)
    nc.vector.tensor_tensor(out=tt[:, :], in0=beta, in1=st[:, :], op=SUB)
    return s, tt


def _stats_allreduce(nc, pool, dram, stats, name, no_collective=False):
    """stats [128,8] -> [128,2] duplicated global sums."""
    red = pool.tile([2 * C, 2], f32, tag=f"red_{name}")
    half = pool.tile([C, 2], f32, tag=f"half_{name}")
    acc = pool.tile([C, 2], f32, tag=f"acc_{name}")
    gstats = pool.tile([2 * C, 2], f32, tag=f"gstats_{name}")
    nc.vector.tensor_reduce(out=red[:, :],
                            in_=stats[:, :].rearrange("p (s c) -> p s c", s=2),
                            axis=mybir.AxisListType.X, op=ADD)
    nc.vector.tensor_copy(out=half[:, :], in_=red[C:2 * C, :])
    nc.vector.tensor_tensor(out=acc[:, :], in0=red[0:C, :], in1=half[:, :],
                            op=ADD)
    if no_collective:
        nc.vector.tensor_copy(out=gstats[0:C, :], in_=acc[:, :])
        nc.vector.tensor_copy(out=gstats[C:2 * C, :], in_=gstats[0:C, :])
        return gstats
    cc_in = dram.tile([C, 2], f32, tag=f"ccin_{name}")
    cc_out = dram.tile([N_CORES * C, 2], f32, tag=f"ccout_{name}")
    gag = pool.tile([C, N_CORES, 2], f32, tag=f"gag_{name}")
    nc.sync.dma_start(out=cc_in[:, :], in_=acc[:, :])
    nc.gpsimd.collective_compute(
        "AllGather", mybir.AluOpType.bypass,
        replica_groups=[list(range(N_CORES))],
        ins=[cc_in.opt()],
        outs=[cc_out.opt()],
    )
    nc.sync.dma_start(
        out=gag[:, :, :],
        in_=cc_out[:, :].rearrange("(r c) s -> c r s", r=N_CORES))
    nc.vector.tensor_reduce(out=gstats[0:C, 0:1], in_=gag[:, :, 0],
                            axis=mybir.AxisListType.X, op=ADD)
    nc.vector.tensor_reduce(out=gstats[0:C, 1:2], in_=gag[:, :, 1],
                            axis=mybir.AxisListType.X, op=ADD)
    nc.vector.tensor_copy(out=gstats[C:2 * C, :], in_=gstats[0:C, :])
    return gstats


def build(no_collective=False, reps=1):
    nc = bacc.Bacc("TRN2", target_bir_lowering=False, debug=False,
                   num_devices=1 if no_collective else N_CORES)
    x_ext = nc.declare_dram_parameter("x", [B_LOCAL, C, HW, HW], f32,
                                      isOutput=False)
    w8_ext = nc.declare_dram_parameter("w8", [2, 128, 5, 2, 128], fp8,
                                       isOutput=False)
    wu_ext = nc.declare_dram_parameter("wu", [128, 128], bf16, isOutput=False)
    cst_ext = nc.declare_dram_parameter("cst", [2 * C, 6], f32, isOutput=False)
    out_ext = nc.declare_dram_parameter("out", [B_LOCAL, C, HW, HW], f32,
                                        isOutput=True)

    n_stat = (B_LOCAL * HW * HW) if no_collective else N_GLOBAL
    with tile.TileContext(nc) as tc:
        with (
            tc.tile_pool(name="big", bufs=1) as big,
            tc.tile_pool(name="xip", bufs=2) as xip,
            tc.tile_pool(name="small", bufs=1) as pool,
            tc.tile_pool(name="psum", bufs=2, space="PSUM") as psum,
            tc.tile_pool(name="dram", bufs=1, space="DRAM") as dram,
        ):
            wx1 = pool.tile([128, 5, 2, 128], fp8, tag="wx1")
            wx2 = pool.tile([128, 5, 2, 128], fp8, tag="wx2")
            wu = pool.tile([128, 128], bf16, tag="wu")
            cst = pool.tile([2 * C, 6], f32, tag="cst")
            g1, b1 = cst[:, 2:3], cst[:, 3:4]
            g2, b2 = cst[:, 4:5], cst[:, 5:6]
            eps = pool.tile([2 * C, 1], f32, tag="eps")
            nc.vector.memset(eps[:, :], BN_EPS)
            nc.gpsimd.dma_start(out=wx1[:, :, :, :], in_=w8_ext[0])
            nc.gpsimd.dma_start(out=wu[:, :], in_=wu_ext[:, :])
            nc.gpsimd.dma_start(out=cst[:, :], in_=cst_ext[:, :])
            nc.gpsimd.dma_start(out=wx2[:, :, :, :], in_=w8_ext[1])

            for r in range(reps):
                # xi: x rows 0..30 at [0:31], x rows 26..55 at [31:61]
                xi = xip.tile([128, NPAIR, 61, HW], f32, tag="xi")
                d = big.tile([128, NPAIR, HW, HW], f32, tag="d")
                F8 = [[big.tile([128, PR, HP], fp8, tag=f"f8_{q}{h}",
                                name=f"f8_{q}{h}") for h in range(2)]
                      for q in range(NPAIR)]
                S = [[big.tile([128, PR, HP], bf16, tag=f"s_{q}{h}",
                               name=f"s_{q}{h}") for h in range(2)]
                     for q in range(NPAIR)]
                H3 = [[big.tile([128, PR, HW], bf16, tag=f"h3_{q}{h}",
                                name=f"h3_{q}{h}") for h in range(2)]
                      for q in range(NPAIR)]
                PYB = [[big.tile([128, PR, HP], bf16, tag=f"yb_{q}{h}",
                                 name=f"yb_{q}{h}") for h in range(2)]
                       for q in range(NPAIR)]
                stats1 = pool.tile([2 * C, 8], f32, tag="stats1")
                stats2 = pool.tile([2 * C, 8], f32, tag="stats2")

                if r == 0:
                    # zero borders/spares once; interior-only writes after.
                    engs = [nc.vector, nc.gpsimd]
                    k = 0
                    for q in range(NPAIR):
                        for h in range(2):
                            for t in (F8[q][h], S[q][h], PYB[q][h]):
                                e = engs[k % 2]
                                k += 1
                                e.memset(t[:, :, :], 0.0)

                # ---- x in: contiguous channel-image runs, 2 DMAs/pair ----
                for q in range(NPAIR):
                    nc.sync.dma_start(
                        out=xi[:, q, 0:31, :],
                        in_=x_ext[2 * q:2 * q + 2, :, 0:31, :]
                            .rearrange("b c r w -> (b c) r w"))
                    nc.scalar.dma_start(
                        out=xi[:, q, 31:61, :],
                        in_=x_ext[2 * q:2 * q + 2, :, 26:56, :]
                            .rearrange("b c r w -> (b c) r w"))

                # ---- L1 prep: F8 = fp8(x), S = x^2, H3 = h-3sum(S) ----
                for q in range(NPAIR):
                    for h in range(2):
                        src = xi[:, q, 0:31, :] if h == 0 else xi[:, q, 31:61, :]
                        f8, s, h3 = F8[q][h], S[q][h], H3[q][h]
                        dst8 = f8[:, 1:32, 1:57] if h == 0 else f8[:, 0:30, 1:57]
                        dsts = s[:, 1:32, 1:57] if h == 0 else s[:, 0:30, 1:57]
                        if CAST_ENGINE == "pool":
                            nc.gpsimd.tensor_copy(out=dst8, in_=src)
                        elif CAST_ENGINE == "dve":
                            nc.vector.tensor_copy(out=dst8, in_=src)
                        else:
                            nc.scalar.activation(out=dst8, in_=src,
                                                 func=AF.Copy)
                        nc.scalar.activation(out=dsts, in_=src, func=AF.Square)
                        nc.vector.tensor_tensor(
                            out=h3[:, :, :], in0=s[:, :, 0:56],
                            in1=s[:, :, 1:57], op=ADD)
                        nc.vector.tensor_tensor(
                            out=h3[:, :, :], in0=h3[:, :, :],
                            in1=s[:, :, 2:58], op=ADD)

                # ---- layer 1 ----
                _layer(nc, psum, F8, H3, wx1, wu, cst, 0, d, stats1)
                gstats1 = _stats_allreduce(nc, pool, dram, stats1,
                                           "l1", no_collective)
                s1, t1 = _bn_affine(nc, pool, gstats1, cst[:, 0:1], g1, b1,
                                    eps, "l1", n_stat=n_stat)

                # ---- glue: y = relu(s1*d + t1) into padded planes ----
                for q in range(NPAIR):
                    for h in range(2):
                        yb, f8, s, h3 = PYB[q][h], F8[q][h], S[q][h], H3[q][h]
                        dsrc = d[:, q, 0:31, :] if h == 0 else d[:, q, 26:56, :]
                        dsty = yb[:, 1:32, 1:57] if h == 0 else yb[:, 0:30, 1:57]
                        dst8 = f8[:, 1:32, 1:57] if h == 0 else f8[:, 0:30, 1:57]
                        ysrc = yb[:, 1:32, 1:57] if h == 0 else yb[:, 0:30, 1:57]
                        nc.scalar.activation(out=dsty, in_=dsrc, func=AF.Relu,
                                             bias=t1[:, 0:1], scale=s1[:, 0:1])
                        if CAST_ENGINE == "pool":
                            nc.gpsimd.tensor_copy(out=dst8, in_=ysrc)
                        elif CAST_ENGINE == "dve":
                            nc.vector.tensor_copy(out=dst8, in_=ysrc)
                        else:
                            nc.scalar.activation(out=dst8, in_=ysrc,
                                                 func=AF.Copy)
                        nc.vector.tensor_tensor(out=s[:, :, :], in0=yb[:, :, :],
                                                in1=yb[:, :, :], op=MULT)
                        nc.vector.tensor_tensor(
                            out=h3[:, :, :], in0=s[:, :, 0:56],
                            in1=s[:, :, 1:57], op=ADD)
                        nc.vector.tensor_tensor(
                            out=h3[:, :, :], in0=h3[:, :, :],
                            in1=s[:, :, 2:58], op=ADD)

                # ---- layer 2 ----
                _layer(nc, psum, F8, H3, wx2, wu, cst, 1, d, stats2)
                gstats2 = _stats_allreduce(nc, pool, dram, stats2,
                                           "l2", no_collective)
                s2, t2 = _bn_affine(nc, pool, gstats2, cst[:, 1:2], g2, b2,
                                    eps, "l2", n_stat=n_stat)

                # ---- final: out = relu(s2*d + t2 + x); contiguous DMA out ----
                for q in range(NPAIR):
                    for h in range(2):
                        rs = slice(h * 28, h * 28 + 28)
                        xs = slice(h * 28, h * 28 + 28) if h == 0 else \
                            slice(33, 61)
                        nc.vector.scalar_tensor_tensor(
                            out=d[:, q, rs, :], in0=d[:, q, rs, :],
                            scalar=s2[:, 0:1], in1=xi[:, q, xs, :],
                            op0=MULT, op1=ADD)
                        nc.scalar.activation(out=d[:, q, rs, :],
                                             in_=d[:, q, rs, :],
                                             func=AF.Relu, bias=t2[:, 0:1],
                                             scale=1.0)
                        eng = nc.sync if h == 0 else nc.scalar
                        eng.dma_start(
                            out=out_ext[2 * q:2 * q + 2, :, rs, :]
                                .rearrange("b c r w -> (b c) r w"),
                            in_=d[:, q, rs, :])
    nc.compile()
    return nc


_NC_CACHE = None


def _get_nc():
    global _NC_CACHE
    if _NC_CACHE is None:
        _NC_CACHE = build()
    return _NC_CACHE


def _make_in_maps(x, centers1, gamma1, beta1, centers2, gamma2, beta2):
    c1 = np.asarray(centers1, np.float32)
    c2 = np.asarray(centers2, np.float32)

    def prep_w8(centers):
        c3 = centers.reshape(C, C, 3, 3)          # (o, ci, kh, kw)
        w = np.zeros((128, 5, 2, 128), np.float32)
        for pi, (_dr, _cb, _jr, offs) in enumerate(FP8_PASSES):
            for j, off in enumerate(offs):
                if off is None:
                    continue
                kh, kw = off
                blk = (-2.0 * SC * c3[:, :, kh, kw]).T   # [ci, o]
                w[0:64, pi, j, 0:64] = blk
                w[64:128, pi, j, 64:128] = blk
        return w.astype(ml_dtypes.float8_e4m3fn)

    w8 = np.stack([prep_w8(c1), prep_w8(c2)], axis=0)
    wu = np.zeros((128, 128), np.float32)
    wu[0:64, 0:64] = SC
    wu[64:128, 64:128] = SC
    wu = wu.astype(ml_dtypes.bfloat16)

    cst = np.stack([
        (c1 ** 2).sum(1), (c2 ** 2).sum(1),
        np.asarray(gamma1, np.float32), np.asarray(beta1, np.float32),
        np.asarray(gamma2, np.float32), np.asarray(beta2, np.float32),
    ], axis=1).astype(np.float32)
    cst = np.ascontiguousarray(np.tile(cst, (2, 1)))
    common = {"w8": w8, "wu": wu, "cst": cst}
    x = np.asarray(x, np.float32)
    in_maps = []
    for c in range(N_CORES):
        m = dict(common)
        m["x"] = np.ascontiguousarray(x[c * B_LOCAL:(c + 1) * B_LOCAL])
        in_maps.append(m)
    return in_maps


def _run(inputs, trace=False, **kw):
    nc = _get_nc()
    in_maps = _make_in_maps(**inputs)
    res = run_bass_kernel_spmd(nc, in_maps, core_ids=list(range(N_CORES)),
                               trace=trace, **kw)
    out = np.concatenate([res.results[c]["out"] for c in range(N_CORES)],
                         axis=0)
    return out.astype(np.float32), res


def kernel(**inputs):
    out, _ = _run(inputs)
    return out
